# revision 5
# baseline (speedup 1.0000x reference)
"""Trainium2 Bass kernel for nn_DynamicResolutionAttention.

B=2, T=2048, C=1024, H=16 heads, head_dim=64.
  q/k/v = x @ W{q,k,v}.T + b     (per-head views)
  attn  = softmax(q k^T / sqrt(hd) * (0.5 + 0.5*resolve))
  y     = attn @ v ; out = y @ Wp.T + bp

Sharding (8 cores): core c = (batch b=c//4, head-group hg=c%4, 4 heads each).
Per core: QKV projections for its 4 heads (transpose-free d-major layouts,
host-pretransposed x^T / W^T), k-major scores S^T = K_h Q_h^T so softmax
denominators come from an appended ones-column on V and no on-chip transpose
is ever needed, exp on the Scalar engine with the runtime temperature,
AllGather of y^T within each batch's 4 cores, then each core computes the
output projection restricted to its own 256 output channels (column split ->
no all-reduce). Host reassembles [B,T,C] from the 8 [T,256] column slices.

Matmul operands are bf16 (fp32 PSUM accumulation); softmax statistics stay fp32.
"""

import sys

for _p in ("/opt/trn_rl_repo",):
    if _p not in sys.path:
        sys.path.insert(0, _p)

import numpy as np

B, T, C, H = 2, 2048, 1024, 16
HD = C // H            # 64
NCORES = 8
HL = 4                 # heads per core
NP = HL // 2           # head pairs per core
CL = HL * HD           # 256 local channels
CIN = C // 128         # 8 contraction tiles
KT_TILES = T // 128    # 16
QC = T // 512          # 4 query chunks

_prog_cache = {}


def _build_program():
    import concourse.mybir as mybir
    import concourse.tile as tile
    from concourse import bacc

    f32 = mybir.dt.float32
    bf16 = mybir.dt.bfloat16

    nc = bacc.Bacc("TRN2", target_bir_lowering=False, debug=False,
                   num_devices=NCORES)

    xT = nc.dram_tensor("xT", [C, T], bf16, kind="ExternalInput")
    wqT = nc.dram_tensor("wqT", [C, CL], bf16, kind="ExternalInput")
    wkT = nc.dram_tensor("wkT", [C, CL], bf16, kind="ExternalInput")
    wvT = nc.dram_tensor("wvT", [C, CL], bf16, kind="ExternalInput")
    wpT = nc.dram_tensor("wpT", [C, CL], bf16, kind="ExternalInput")
    bq = nc.dram_tensor("bq", [1, CL], bf16, kind="ExternalInput")
    bk = nc.dram_tensor("bk", [1, CL], bf16, kind="ExternalInput")
    bv = nc.dram_tensor("bv", [1, CL], bf16, kind="ExternalInput")
    bp = nc.dram_tensor("bp", [1, CL], bf16, kind="ExternalInput")
    rlv = nc.dram_tensor("rlv", [1, 1], f32, kind="ExternalInput")
    ones_d = nc.dram_tensor("ones_d", [1, 512], bf16, kind="ExternalInput")
    z = nc.dram_tensor("z", [T, CL], f32, kind="ExternalOutput")

    with tile.TileContext(nc) as tc:
        with tc.tile_pool(name="const", bufs=1) as const, \
             tc.tile_pool(name="big", bufs=1) as big, \
             tc.tile_pool(name="xp", bufs=2) as xp, \
             tc.tile_pool(name="work", bufs=3) as work, \
             tc.tile_pool(name="ps", bufs=2, space="PSUM") as ps, \
             tc.tile_pool(name="dram", bufs=1, space="DRAM") as dram:

            # runtime softmax scale: (0.5 + 0.5*resolve) / sqrt(hd)
            st = const.tile([128, 1], f32)
            nc.sync.dma_start(st[:], rlv[:].to_broadcast((128, 1)))
            nc.vector.tensor_scalar(st[:], st[:], 0.0625, 0.0625,
                                    mybir.AluOpType.mult, mybir.AluOpType.add)

            ones512 = const.tile([1, 512], bf16)
            nc.sync.dma_start(ones512[:], ones_d[:])
            ones128 = const.tile([1, 128], bf16)
            nc.sync.dma_start(ones128[:], ones_d[:, 0:128])
            ones64 = const.tile([1, 64], f32)
            nc.vector.memset(ones64[:], 1.0)

            bq_sb = const.tile([1, CL], bf16)
            bk_sb = const.tile([1, CL], bf16)
            bv_sb = const.tile([1, CL], bf16)
            bp_sb = const.tile([1, CL], bf16)
            nc.sync.dma_start(bq_sb[:], bq[:])
            nc.sync.dma_start(bk_sb[:], bk[:])
            nc.sync.dma_start(bv_sb[:], bv[:])
            nc.sync.dma_start(bp_sb[:], bp[:])

            wq_sb = big.tile([128, CIN, CL], bf16)
            wk_sb = big.tile([128, CIN, CL], bf16)
            wv_sb = big.tile([128, CIN, CL], bf16)
            wp_sb = big.tile([128, CIN, CL], bf16)
            for w_sb, w_dram in ((wq_sb, wqT), (wk_sb, wkT), (wv_sb, wvT)):
                w3 = w_dram[:].rearrange("(o p) c -> p o c", p=128)
                for ci in range(CIN):
                    nc.sync.dma_start(w_sb[:, ci, :], w3[:, ci, :])

            QT = big.tile([128, NP, T], bf16)
            KT = big.tile([128, NP, T], bf16)
            V = big.tile([128, KT_TILES, HL, HD + 1], bf16)
            nc.sync.dma_start(
                V[:, :, :, HD].rearrange("p a b -> p (a b)"),
                ones_d[0:1, 0:1].to_broadcast((128, KT_TILES * HL)))

            # ---- phase 1: QKV projections, processed in two k-halves ----
            xT3 = xT[:].rearrange("(o p) t -> p o t", p=128)
            for kh in range(2):
                t0 = kh * (T // 2)
                xs = xp.tile([128, CIN, T // 2], bf16, tag="xT")
                for ci in range(CIN):
                    nc.sync.dma_start(xs[:, ci, :], xT3[:, ci, t0:t0 + T // 2])

                for w_sb, b_sb, OUT in ((wq_sb, bq_sb, QT), (wk_sb, bk_sb, KT)):
                    for pair in range(NP):
                        pc = slice(pair * 128, (pair + 1) * 128)
                        for ch in range(2):
                            pm = ps.tile([128, 512], f32, tag="mm")
                            nc.tensor.matmul(pm[:], b_sb[:, pc], ones512[:],
                                             start=True, stop=False)
                            for ci in range(CIN):
                                nc.tensor.matmul(
                                    pm[:], w_sb[:, ci, pc],
                                    xs[:, ci, ch * 512:(ch + 1) * 512],
                                    start=False, stop=(ci == CIN - 1))
                            dst = OUT[:, pair,
                                      t0 + ch * 512:t0 + (ch + 1) * 512]
                            if OUT is QT:
                                # fold softmax temperature into Q
                                nc.vector.tensor_scalar_mul(dst, pm[:], st[:])
                            else:
                                nc.vector.tensor_copy(dst, pm[:])

                for tt in range(8):
                    pv = ps.tile([128, CL], f32, tag="mm")
                    nc.tensor.matmul(pv[:], ones128[:], bv_sb[:],
                                     start=True, stop=False)
                    for ci in range(CIN):
                        nc.tensor.matmul(
                            pv[:], xs[:, ci, tt * 128:(tt + 1) * 128],
                            wv_sb[:, ci, :],
                            start=False, stop=(ci == CIN - 1))
                    nc.vector.tensor_copy(
                        V[:, kh * 8 + tt, :, 0:HD],
                        pv[:].rearrange("p (h d) -> p h d", h=HL))

            # wp loads can overlap attention
            wp3 = wpT[:].rearrange("(o p) c -> p o c", p=128)
            for ci in range(CIN):
                nc.sync.dma_start(wp_sb[:, ci, :], wp3[:, ci, :])

            ag_in = dram.tile([CL, T], bf16)
            ag_out = dram.tile([4, CL, T], bf16)

            # ---- phase 2: attention (k-major S^T, ones-column denominators) --
            for h in range(HL):
                pair, off = h // 2, (h % 2) * HD
                for qc in range(QC):
                    qs = slice(qc * 512, (qc + 1) * 512)
                    py = ps.tile([HD + 1, 512], f32, tag="y")
                    for kt in range(KT_TILES):
                        pss = ps.tile([128, 512], f32, tag="s", bufs=3)
                        nc.tensor.matmul(
                            pss[:],
                            KT[off:off + HD, pair, kt * 128:(kt + 1) * 128],
                            QT[off:off + HD, pair, qs],
                            start=True, stop=True)
                        pt = work.tile([128, 512], bf16, tag="pt", bufs=6)
                        nc.scalar.activation(pt[:], pss[:],
                                             mybir.ActivationFunctionType.Exp)
                        nc.tensor.matmul(py[:], V[:, kt, h, :], pt[:],
                                         start=(kt == 0),
                                         stop=(kt == KT_TILES - 1))
                    rec = work.tile([1, 512], f32, tag="rec")
                    nc.vector.reciprocal(rec[:], py[HD:HD + 1, :])
                    pb = ps.tile([HD, 512], f32, tag="b", bufs=1)
                    nc.tensor.matmul(pb[:], ones64[:], rec[:],
                                     start=True, stop=True)
                    pbs = work.tile([HD, 512], f32, tag="pbs")
                    nc.vector.tensor_copy(pbs[:], pb[:])
                    yt = work.tile([HD, 512], bf16, tag="yt")
                    nc.vector.tensor_mul(yt[:], py[0:HD, :], pbs[:])
                    nc.sync.dma_start(ag_in[h * HD:(h + 1) * HD, qs], yt[:])

            # ---- phase 3: gather all heads' y^T within the batch group ----
            nc.gpsimd.collective_compute(
                "AllGather", mybir.AluOpType.bypass,
                replica_groups=[[0, 1, 2, 3], [4, 5, 6, 7]],
                ins=[ag_in.opt()], outs=[ag_out.opt()])

            # ---- phase 4: output projection, this core's 256 columns ----
            ag_flat = ag_out[:].rearrange("g c t -> (g c) t") \
                               .rearrange("(o p) t -> p o t", p=128)
            for tt in range(KT_TILES):
                ts = slice(tt * 128, (tt + 1) * 128)
                ys = work.tile([128, CIN, 128], bf16, tag="ys")
                nc.sync.dma_start(ys[:], ag_flat[:, :, ts])
                pz = ps.tile([128, CL], f32, tag="mm")
                nc.tensor.matmul(pz[:], ones128[:], bp_sb[:],
                                 start=True, stop=False)
                for ci in range(CIN):
                    nc.tensor.matmul(pz[:], ys[:, ci, :], wp_sb[:, ci, :],
                                     start=False, stop=(ci == CIN - 1))
                zs = work.tile([128, CL], f32, tag="zs")
                nc.vector.tensor_copy(zs[:], pz[:])
                nc.sync.dma_start(z[ts, :], zs[:])

    nc.compile()
    return nc


def _get_program():
    if "nc" not in _prog_cache:
        _prog_cache["nc"] = _build_program()
    return _prog_cache["nc"]


def kernel(x, Wq, bq, Wk, bk, Wv, bv, Wp, bp, resolve_level):
    import ml_dtypes
    from concourse.bass_utils import run_bass_kernel_spmd

    bfl = ml_dtypes.bfloat16
    nc = _get_program()

    x = np.asarray(x, np.float32)
    rl = np.asarray(resolve_level, np.float32).reshape(1, 1)

    xT_b = [np.ascontiguousarray(x[b].T).astype(bfl) for b in range(B)]
    in_maps = []
    for c in range(NCORES):
        b, hg = c // 4, c % 4
        cs = slice(hg * CL, (hg + 1) * CL)
        in_maps.append({
            "xT": xT_b[b],
            "wqT": np.ascontiguousarray(np.asarray(Wq, np.float32)[cs, :].T).astype(bfl),
            "wkT": np.ascontiguousarray(np.asarray(Wk, np.float32)[cs, :].T).astype(bfl),
            "wvT": np.ascontiguousarray(np.asarray(Wv, np.float32)[cs, :].T).astype(bfl),
            "wpT": np.ascontiguousarray(np.asarray(Wp, np.float32)[cs, :].T).astype(bfl),
            "bq": np.asarray(bq, np.float32)[cs].reshape(1, CL).astype(bfl),
            "bk": np.asarray(bk, np.float32)[cs].reshape(1, CL).astype(bfl),
            "bv": np.asarray(bv, np.float32)[cs].reshape(1, CL).astype(bfl),
            "bp": np.asarray(bp, np.float32)[cs].reshape(1, CL).astype(bfl),
            "rlv": rl,
            "ones_d": np.ones((1, 512), bfl),
        })

    res = run_bass_kernel_spmd(nc, in_maps, core_ids=list(range(NCORES)))

    out = np.empty((B, T, C), np.float32)
    for c in range(NCORES):
        b, hg = c // 4, c % 4
        out[b, :, hg * CL:(hg + 1) * CL] = res.results[c]["z"]
    return out


# revision 8
# speedup vs baseline: 1.6218x; 1.6218x over previous
"""Trainium2 Bass kernel for nn_DynamicResolutionAttention.

B=2, T=2048, C=1024, H=16 heads, head_dim=64.
  q/k/v = x @ W{q,k,v}.T + b     (per-head views)
  attn  = softmax(q k^T / sqrt(hd) * (0.5 + 0.5*resolve))
  y     = attn @ v ; out = y @ Wp.T + bp

Sharding (8 cores): core c = (batch b=c//4, head-group hg=c%4, 4 heads each).
Per core: QKV projections for its 4 heads (transpose-free d-major layouts,
host-pretransposed x^T / W^T in partition-major order for wide DMA lines),
k-major scores S^T = K_h Q_h^T so softmax denominators come from an appended
ones-column on V and no on-chip transpose is ever needed, exp on the Scalar
engine (softmax temperature pre-folded into Q), AllGather of y^T within each
batch's 4 cores (split into two token-halves so the first gather overlaps the
second half of attention), then each core computes the output projection
restricted to its own 256 output channels (column split -> no all-reduce).
Host reassembles [B,T,C] from the 8 [T,256] column slices.

Matmul operands are bf16 (fp32 PSUM accumulation); softmax statistics fp32.
"""

import sys

for _p in ("/opt/trn_rl_repo",):
    if _p not in sys.path:
        sys.path.insert(0, _p)

import numpy as np

B, T, C, H = 2, 2048, 1024, 16
HD = C // H            # 64
NCORES = 8
HL = 4                 # heads per core
NP = HL // 2           # head pairs per core
CL = HL * HD           # 256 local channels
CIN = C // 128         # 8 contraction tiles
KT_TILES = T // 128    # 16
QC = T // 512          # 4 query chunks
TH = T // 2            # token half

_prog_cache = {}


def _build_program():
    import concourse.mybir as mybir
    import concourse.tile as tile
    from concourse import bacc

    f32 = mybir.dt.float32
    bf16 = mybir.dt.bfloat16

    nc = bacc.Bacc("TRN2", target_bir_lowering=False, debug=False,
                   num_devices=NCORES)

    # host-prearranged partition-major layouts (long contiguous DMA lines)
    xP = nc.dram_tensor("xP", [128, CIN, T], bf16, kind="ExternalInput")
    wqP = nc.dram_tensor("wqP", [128, CIN, CL], bf16, kind="ExternalInput")
    wkP = nc.dram_tensor("wkP", [128, CIN, CL], bf16, kind="ExternalInput")
    wvP = nc.dram_tensor("wvP", [128, CIN, CL], bf16, kind="ExternalInput")
    wpP = nc.dram_tensor("wpP", [128, CIN, CL], bf16, kind="ExternalInput")
    bqC = nc.dram_tensor("bqC", [128, NP], f32, kind="ExternalInput")
    bkC = nc.dram_tensor("bkC", [128, NP], f32, kind="ExternalInput")
    bv = nc.dram_tensor("bv", [1, CL], bf16, kind="ExternalInput")
    bp = nc.dram_tensor("bp", [1, CL], bf16, kind="ExternalInput")
    rlv = nc.dram_tensor("rlv", [1, 1], f32, kind="ExternalInput")
    ones_d = nc.dram_tensor("ones_d", [1, 512], bf16, kind="ExternalInput")
    z = nc.dram_tensor("z", [T, CL], f32, kind="ExternalOutput")

    with tile.TileContext(nc) as tc:
        with tc.tile_pool(name="const", bufs=1) as const, \
             tc.tile_pool(name="big", bufs=1) as big, \
             tc.tile_pool(name="xp", bufs=2) as xp, \
             tc.tile_pool(name="work", bufs=3) as work, \
             tc.tile_pool(name="ps", bufs=2, space="PSUM") as ps, \
             tc.tile_pool(name="dram", bufs=1, space="DRAM") as dram:

            # runtime softmax scale: (0.5 + 0.5*resolve) / sqrt(hd)
            st = const.tile([128, 1], f32)
            nc.sync.dma_start(st[:], rlv[:].to_broadcast((128, 1)))
            nc.vector.tensor_scalar(st[:], st[:], 0.0625, 0.0625,
                                    mybir.AluOpType.mult, mybir.AluOpType.add)

            ones512 = const.tile([1, 512], bf16)
            nc.sync.dma_start(ones512[:], ones_d[:])
            ones128 = const.tile([1, 128], bf16)
            nc.sync.dma_start(ones128[:], ones_d[:, 0:128])
            ones64 = const.tile([1, 64], f32)
            nc.vector.memset(ones64[:], 1.0)

            bqC_sb = const.tile([128, NP], f32)
            bkC_sb = const.tile([128, NP], f32)
            bv_sb = const.tile([1, CL], bf16)
            bp_sb = const.tile([1, CL], bf16)
            nc.sync.dma_start(bqC_sb[:], bqC[:])
            nc.sync.dma_start(bkC_sb[:], bkC[:])
            nc.sync.dma_start(bv_sb[:], bv[:])
            nc.sync.dma_start(bp_sb[:], bp[:])

            # x resident for phase 1: 8 parallel DMAs, 4KB lines
            xs = xp.tile([128, CIN, T], bf16, tag="xT")
            for ci in range(CIN):
                eng = nc.sync if ci % 2 == 0 else nc.scalar
                eng.dma_start(xs[:, ci, :], xP[:, ci, :])

            wq_sb = big.tile([128, CIN, CL], bf16)
            wk_sb = big.tile([128, CIN, CL], bf16)
            wv_sb = big.tile([128, CIN, CL], bf16)
            wp_sb = big.tile([128, CIN, CL], bf16)
            for i, (w_sb, w_dram) in enumerate(
                    ((wq_sb, wqP), (wk_sb, wkP), (wv_sb, wvP))):
                eng = nc.scalar if i % 2 == 0 else nc.sync
                eng.dma_start(w_sb[:, 0:4, :], w_dram[:, 0:4, :])
                eng.dma_start(w_sb[:, 4:8, :], w_dram[:, 4:8, :])

            QTp = [big.tile([128, T], bf16, name=f"QT{p}") for p in range(NP)]
            KTp = [big.tile([128, T], bf16, name=f"KT{p}") for p in range(NP)]
            Vp = [big.tile([128, KT_TILES, 2, HD + 1], bf16, name=f"V{p}")
                  for p in range(NP)]
            for p in range(NP):
                nc.sync.dma_start(
                    Vp[p][:, :, :, HD].rearrange("p a b -> p (a b)"),
                    ones_d[0:1, 0:1].to_broadcast((128, KT_TILES * 2)))

            # ---- phase 1: QKV projections (pair 0 first for early attn) --
            def qk_proj(pair):
                pc = slice(pair * 128, (pair + 1) * 128)
                for w_sb, OUT, bc in ((wq_sb, QTp[pair], bqC_sb),
                                      (wk_sb, KTp[pair], bkC_sb)):
                    for ch in range(QC):
                        pm = ps.tile([128, 2, 512], f32, tag="s",
                                     name="pm")
                        pm = pm[:, 0, :]
                        for ci in range(CIN):
                            nc.tensor.matmul(
                                pm, w_sb[:, ci, pc],
                                xs[:, ci, ch * 512:(ch + 1) * 512],
                                start=(ci == 0), stop=(ci == CIN - 1))
                        dst = OUT[:, ch * 512:(ch + 1) * 512]
                        if OUT is QTp[pair]:
                            # (q + bias) * temperature
                            nc.vector.tensor_scalar(
                                dst, pm, bc[:, pair:pair + 1], st[:],
                                mybir.AluOpType.add, mybir.AluOpType.mult)
                        else:
                            nc.vector.tensor_scalar_add(
                                dst, pm, bc[:, pair:pair + 1])

            def v_proj():
                for tt in range(KT_TILES):
                    pv = ps.tile([128, 2, 512], f32, tag="s", name="pv")
                    pv = pv[:, 0, 0:CL]
                    nc.tensor.matmul(pv, ones128[:], bv_sb[:],
                                     start=True, stop=False)
                    for ci in range(CIN):
                        nc.tensor.matmul(
                            pv, xs[:, ci, tt * 128:(tt + 1) * 128],
                            wv_sb[:, ci, :],
                            start=False, stop=(ci == CIN - 1))
                    for p in range(NP):
                        nc.vector.tensor_copy(
                            Vp[p][:, tt, :, 0:HD],
                            pv[:, p * 128:(p + 1) * 128]
                            .rearrange("p (h d) -> p h d", h=2))

            qk_proj(0)
            v_proj()
            qk_proj(1)

            # wp loads overlap attention
            nc.scalar.dma_start(wp_sb[:, 0:4, :], wpP[:, 0:4, :])
            nc.scalar.dma_start(wp_sb[:, 4:8, :], wpP[:, 4:8, :])

            ag_in = [dram.tile([CL, 512], bf16, name=f"ag_in{i}")
                     for i in range(QC)]
            ag_out = [dram.tile([4, CL, 512], bf16, name=f"ag_out{i}")
                      for i in range(QC)]

            # ---- phase 2: attention (k-major S^T, ones-column denom) ----
            # qc outer so each token-quarter's AllGather can start early
            for qc in range(QC):
                qs = slice(qc * 512, (qc + 1) * 512)
                for h in range(HL):
                    pair, off = h // 2, (h % 2) * HD
                    hh = h % 2
                    QT_, KT_, V_ = QTp[pair], KTp[pair], Vp[pair]
                    py = ps.tile([HD + 1, 512], f32, tag="y", name="py")
                    for k2 in range(KT_TILES // 2):
                        pss = ps.tile([128, 2, 512], f32, tag="s",
                                      name="pss")
                        for j in range(2):
                            kt = k2 * 2 + j
                            nc.tensor.matmul(
                                pss[:, j, :],
                                KT_[off:off + HD,
                                    kt * 128:(kt + 1) * 128],
                                QT_[off:off + HD, qs],
                                start=True, stop=True)
                        pt = work.tile([128, 2, 512], bf16, tag="pt", bufs=6)
                        nc.scalar.activation(
                            pt[:], pss[:],
                            mybir.ActivationFunctionType.Exp)
                        for j in range(2):
                            kt = k2 * 2 + j
                            nc.tensor.matmul(
                                py[:], V_[:, kt, hh, :], pt[:, j, :],
                                start=(kt == 0),
                                stop=(kt == KT_TILES - 1))
                    rec = work.tile([1, 512], f32, tag="rec")
                    nc.vector.reciprocal(rec[:], py[HD:HD + 1, :])
                    pb = ps.tile([HD, 512], f32, tag="b", bufs=1)
                    nc.tensor.matmul(pb[:], ones64[:], rec[:],
                                     start=True, stop=True)
                    pbs = work.tile([HD, 512], f32, tag="pbs")
                    nc.vector.tensor_copy(pbs[:], pb[:])
                    yt = work.tile([HD, 512], bf16, tag="yt")
                    nc.vector.tensor_mul(yt[:], py[0:HD, :], pbs[:])
                    nc.sync.dma_start(
                        ag_in[qc][h * HD:(h + 1) * HD, :], yt[:])

                # ---- phase 3: gather heads' y^T for this token quarter ----
                nc.gpsimd.collective_compute(
                    "AllGather", mybir.AluOpType.bypass,
                    replica_groups=[[0, 1, 2, 3], [4, 5, 6, 7]],
                    ins=[ag_in[qc].opt()], outs=[ag_out[qc].opt()])

            # ---- phase 4: output projection (this core's 256 columns) ----
            for qc in range(QC):
                ysb = xp.tile([128, CIN, 512], bf16, tag="xT", name="ysb")
                agf = ag_out[qc][:].rearrange("g c t -> (g c) t") \
                                   .rearrange("(o p) t -> p o t", p=128)
                for ci in range(CIN):
                    eng = nc.sync if ci % 2 == 0 else nc.scalar
                    eng.dma_start(ysb[:, ci, :], agf[:, ci, :])
                for tt in range(4):
                    pz = ps.tile([128, 2, 512], f32, tag="s", name="pz")
                    pz = pz[:, 0, 0:CL]
                    nc.tensor.matmul(pz, ones128[:], bp_sb[:],
                                     start=True, stop=False)
                    for ci in range(CIN):
                        nc.tensor.matmul(
                            pz, ysb[:, ci, tt * 128:(tt + 1) * 128],
                            wp_sb[:, ci, :],
                            start=False, stop=(ci == CIN - 1))
                    zs = work.tile([128, CL], f32, tag="zs")
                    nc.vector.tensor_copy(zs[:], pz)
                    t0 = qc * 512 + tt * 128
                    nc.scalar.dma_start(z[t0:t0 + 128, :], zs[:])

    nc.compile()
    return nc


def _get_program():
    if "nc" not in _prog_cache:
        _prog_cache["nc"] = _build_program()
    return _prog_cache["nc"]


def _pmajor(a2d):
    """[C, N] -> [128, C//128, N] partition-major contiguous."""
    Cdim, N = a2d.shape
    return np.ascontiguousarray(
        a2d.reshape(CIN, 128, N).transpose(1, 0, 2))


def kernel(x, Wq, bq, Wk, bk, Wv, bv, Wp, bp, resolve_level):
    import ml_dtypes
    from concourse.bass_utils import run_bass_kernel_spmd

    bfl = ml_dtypes.bfloat16
    nc = _get_program()

    x = np.asarray(x, np.float32)
    rl = np.asarray(resolve_level, np.float32).reshape(1, 1)

    xP_b = [_pmajor(np.ascontiguousarray(x[b].T).astype(bfl))
            for b in range(B)]
    in_maps = []
    for c in range(NCORES):
        b, hg = c // 4, c % 4
        cs = slice(hg * CL, (hg + 1) * CL)
        in_maps.append({
            "xP": xP_b[b],
            "wqP": _pmajor(np.asarray(Wq, np.float32)[cs, :].T.astype(bfl)),
            "wkP": _pmajor(np.asarray(Wk, np.float32)[cs, :].T.astype(bfl)),
            "wvP": _pmajor(np.asarray(Wv, np.float32)[cs, :].T.astype(bfl)),
            "wpP": _pmajor(np.asarray(Wp, np.float32)[cs, :].T.astype(bfl)),
            "bqC": np.ascontiguousarray(
                np.asarray(bq, np.float32)[cs].reshape(NP, 128).T),
            "bkC": np.ascontiguousarray(
                np.asarray(bk, np.float32)[cs].reshape(NP, 128).T),
            "bv": np.asarray(bv, np.float32)[cs].reshape(1, CL).astype(bfl),
            "bp": np.asarray(bp, np.float32)[cs].reshape(1, CL).astype(bfl),
            "rlv": rl,
            "ones_d": np.ones((1, 512), bfl),
        })

    res = run_bass_kernel_spmd(nc, in_maps, core_ids=list(range(NCORES)))

    out = np.empty((B, T, C), np.float32)
    for c in range(NCORES):
        b, hg = c // 4, c % 4
        out[b, :, hg * CL:(hg + 1) * CL] = res.results[c]["z"]
    return out


# revision 10
# speedup vs baseline: 1.7778x; 1.0962x over previous
"""Trainium2 Bass kernel for nn_DynamicResolutionAttention.

B=2, T=2048, C=1024, H=16 heads, head_dim=64.
  q/k/v = x @ W{q,k,v}.T + b     (per-head views)
  attn  = softmax(q k^T / sqrt(hd) * (0.5 + 0.5*resolve))
  y     = attn @ v ; out = y @ Wp.T + bp

Sharding (8 cores): core c = (batch b=c//4, head-group hg=c%4, 4 heads each).
Per core: QKV projections for its 4 heads (transpose-free d-major layouts,
host-pretransposed x^T / W^T in partition-major order for wide DMA lines),
k-major scores S^T = K_h Q_h^T so softmax denominators come from an appended
ones-column on V and no on-chip transpose is ever needed, exp on the Scalar
engine (softmax temperature pre-folded into Q), AllGather of y^T within each
batch's 4 cores (split into two token-halves so the first gather overlaps the
second half of attention), then each core computes the output projection
restricted to its own 256 output channels (column split -> no all-reduce).
Host reassembles [B,T,C] from the 8 [T,256] column slices.

Matmul operands are bf16 (fp32 PSUM accumulation); softmax statistics fp32.
"""

import sys

for _p in ("/opt/trn_rl_repo",):
    if _p not in sys.path:
        sys.path.insert(0, _p)

import numpy as np

B, T, C, H = 2, 2048, 1024, 16
HD = C // H            # 64
NCORES = 8
HL = 4                 # heads per core
NP = HL // 2           # head pairs per core
CL = HL * HD           # 256 local channels
CIN = C // 128         # 8 contraction tiles
KT_TILES = T // 128    # 16
QC = T // 512          # 4 query chunks
TH = T // 2            # token half

_prog_cache = {}


def _build_program():
    import concourse.mybir as mybir
    import concourse.tile as tile
    from concourse import bacc

    f32 = mybir.dt.float32
    bf16 = mybir.dt.bfloat16

    nc = bacc.Bacc("TRN2", target_bir_lowering=False, debug=False,
                   num_devices=NCORES)

    # host-prearranged partition-major layouts (long contiguous DMA lines)
    xP = nc.dram_tensor("xP", [128, CIN, T], bf16, kind="ExternalInput")
    wqP = nc.dram_tensor("wqP", [128, CIN, CL], bf16, kind="ExternalInput")
    wkP = nc.dram_tensor("wkP", [128, CIN, CL], bf16, kind="ExternalInput")
    wvP = nc.dram_tensor("wvP", [128, CIN, CL], bf16, kind="ExternalInput")
    wpP = nc.dram_tensor("wpP", [128, CIN, CL], bf16, kind="ExternalInput")
    bqC = nc.dram_tensor("bqC", [128, NP], f32, kind="ExternalInput")
    bkC = nc.dram_tensor("bkC", [128, NP], f32, kind="ExternalInput")
    bv = nc.dram_tensor("bv", [1, CL], bf16, kind="ExternalInput")
    bp = nc.dram_tensor("bp", [1, CL], bf16, kind="ExternalInput")
    rlv = nc.dram_tensor("rlv", [1, 1], f32, kind="ExternalInput")
    ones_d = nc.dram_tensor("ones_d", [1, 512], bf16, kind="ExternalInput")
    z = nc.dram_tensor("z", [T, CL], f32, kind="ExternalOutput")

    with tile.TileContext(nc) as tc:
        with tc.tile_pool(name="const", bufs=1) as const, \
             tc.tile_pool(name="big", bufs=1) as big, \
             tc.tile_pool(name="xp", bufs=2) as xp, \
             tc.tile_pool(name="work", bufs=3) as work, \
             tc.tile_pool(name="ps", bufs=2, space="PSUM") as ps, \
             tc.tile_pool(name="dram", bufs=1, space="DRAM") as dram:

            # runtime softmax scale: (0.5 + 0.5*resolve) / sqrt(hd)
            st = const.tile([128, 1], f32)
            nc.sync.dma_start(st[:], rlv[:].to_broadcast((128, 1)))
            nc.vector.tensor_scalar(st[:], st[:], 0.0625, 0.0625,
                                    mybir.AluOpType.mult, mybir.AluOpType.add)

            ones512 = const.tile([1, 512], bf16)
            nc.sync.dma_start(ones512[:], ones_d[:])
            ones128 = const.tile([1, 128], bf16)
            nc.sync.dma_start(ones128[:], ones_d[:, 0:128])

            bqC_sb = const.tile([128, NP], f32)
            bkC_sb = const.tile([128, NP], f32)
            bv_sb = const.tile([1, CL], bf16)
            bp_sb = const.tile([1, CL], bf16)
            nc.sync.dma_start(bqC_sb[:], bqC[:])
            nc.sync.dma_start(bkC_sb[:], bkC[:])
            nc.sync.dma_start(bv_sb[:], bv[:])
            nc.sync.dma_start(bp_sb[:], bp[:])

            # x resident for phase 1: 8 parallel DMAs, 4KB lines
            xs = xp.tile([128, CIN, T], bf16, tag="xT")
            for ci in range(CIN):
                for hf in range(2):
                    eng = nc.sync if (ci * 2 + hf) % 2 == 0 else nc.scalar
                    eng.dma_start(xs[:, ci, hf * TH:(hf + 1) * TH],
                                  xP[:, ci, hf * TH:(hf + 1) * TH])

            wq_sb = big.tile([128, CIN, CL], bf16)
            wk_sb = big.tile([128, CIN, CL], bf16)
            wv_sb = big.tile([128, CIN, CL], bf16)
            wp_sb = big.tile([128, CIN, CL], bf16)
            for i, (w_sb, w_dram) in enumerate(
                    ((wq_sb, wqP), (wk_sb, wkP), (wv_sb, wvP))):
                eng = nc.scalar if i % 2 == 0 else nc.sync
                eng.dma_start(w_sb[:, 0:4, :], w_dram[:, 0:4, :])
                eng.dma_start(w_sb[:, 4:8, :], w_dram[:, 4:8, :])

            QTp = [big.tile([128, T], bf16, name=f"QT{p}") for p in range(NP)]
            KTp = [big.tile([128, T], bf16, name=f"KT{p}") for p in range(NP)]
            Vp = [big.tile([128, KT_TILES, 2, HD + 1], bf16, name=f"V{p}")
                  for p in range(NP)]
            for p in range(NP):
                nc.sync.dma_start(
                    Vp[p][:, :, :, HD].rearrange("p a b -> p (a b)"),
                    ones_d[0:1, 0:1].to_broadcast((128, KT_TILES * 2)))

            # ---- phase 1: QKV projections (pair 0 first for early attn) --
            def qk_proj(pair):
                pc = slice(pair * 128, (pair + 1) * 128)
                for w_sb, OUT, bc in ((wq_sb, QTp[pair], bqC_sb),
                                      (wk_sb, KTp[pair], bkC_sb)):
                    for ch in range(QC):
                        pm = ps.tile([128, 2, 512], f32, tag="s",
                                     name="pm", bufs=3)
                        pm = pm[:, 0, :]
                        for ci in range(CIN):
                            nc.tensor.matmul(
                                pm, w_sb[:, ci, pc],
                                xs[:, ci, ch * 512:(ch + 1) * 512],
                                start=(ci == 0), stop=(ci == CIN - 1))
                        dst = OUT[:, ch * 512:(ch + 1) * 512]
                        if OUT is QTp[pair]:
                            # (q + bias) * temperature
                            nc.vector.tensor_scalar(
                                dst, pm, bc[:, pair:pair + 1], st[:],
                                mybir.AluOpType.add, mybir.AluOpType.mult)
                        else:
                            nc.vector.tensor_scalar_add(
                                dst, pm, bc[:, pair:pair + 1])

            def v_proj():
                for tt in range(KT_TILES):
                    pv = ps.tile([128, 2, 512], f32, tag="s", name="pv", bufs=3)
                    pv = pv[:, 0, 0:CL]
                    nc.tensor.matmul(pv, ones128[:], bv_sb[:],
                                     start=True, stop=False)
                    for ci in range(CIN):
                        nc.tensor.matmul(
                            pv, xs[:, ci, tt * 128:(tt + 1) * 128],
                            wv_sb[:, ci, :],
                            start=False, stop=(ci == CIN - 1))
                    for p in range(NP):
                        nc.vector.tensor_copy(
                            Vp[p][:, tt, :, 0:HD],
                            pv[:, p * 128:(p + 1) * 128]
                            .rearrange("p (h d) -> p h d", h=2))

            qk_proj(0)
            v_proj()
            qk_proj(1)

            # wp loads overlap attention
            nc.scalar.dma_start(wp_sb[:, 0:4, :], wpP[:, 0:4, :])
            nc.scalar.dma_start(wp_sb[:, 4:8, :], wpP[:, 4:8, :])

            ag_in = [dram.tile([CL, 512], bf16, name=f"ag_in{i}")
                     for i in range(QC)]
            rec_d = dram.tile([16, 512], f32, name="rec_d")
            ag_out = [dram.tile([4, CL, 512], bf16, name=f"ag_out{i}")
                      for i in range(QC)]

            # ---- phase 2: attention (k-major S^T, ones-column denom) ----
            # qc outer so each token-quarter's AllGather can start early
            for qc in range(QC):
                qs = slice(qc * 512, (qc + 1) * 512)
                for h in range(HL):
                    pair, off = h // 2, (h % 2) * HD
                    hh = h % 2
                    QT_, KT_, V_ = QTp[pair], KTp[pair], Vp[pair]
                    py = ps.tile([HD + 1, 512], f32, tag="y", name="py")
                    for k2 in range(KT_TILES // 2):
                        pss = ps.tile([128, 2, 512], f32, tag="s",
                                      name="pss", bufs=3)
                        for j in range(2):
                            kt = k2 * 2 + j
                            nc.tensor.matmul(
                                pss[:, j, :],
                                KT_[off:off + HD,
                                    kt * 128:(kt + 1) * 128],
                                QT_[off:off + HD, qs],
                                start=True, stop=True)
                        pt = work.tile([128, 2, 512], bf16, tag="pt", bufs=6)
                        nc.scalar.activation(
                            pt[:], pss[:],
                            mybir.ActivationFunctionType.Exp)
                        for j in range(2):
                            kt = k2 * 2 + j
                            nc.tensor.matmul(
                                py[:], V_[:, kt, hh, :], pt[:, j, :],
                                start=(kt == 0),
                                stop=(kt == KT_TILES - 1))
                    rec = work.tile([1, 512], f32, tag="rec")
                    nc.vector.reciprocal(rec[:], py[HD:HD + 1, :])
                    slot = qc * HL + h
                    nc.sync.dma_start(rec_d[slot:slot + 1, :], rec[:])
                    pbs = work.tile([HD, 512], f32, tag="pbs")
                    nc.sync.dma_start(
                        pbs[:], rec_d[slot:slot + 1, :].to_broadcast((HD, 512)))
                    yt = work.tile([HD, 512], bf16, tag="yt")
                    nc.vector.tensor_mul(yt[:], py[0:HD, :], pbs[:])
                    nc.sync.dma_start(
                        ag_in[qc][h * HD:(h + 1) * HD, :], yt[:])

                # ---- phase 3: gather heads' y^T for this token quarter ----
                nc.gpsimd.collective_compute(
                    "AllGather", mybir.AluOpType.bypass,
                    replica_groups=[[0, 1, 2, 3], [4, 5, 6, 7]],
                    ins=[ag_in[qc].opt()], outs=[ag_out[qc].opt()])

            # ---- phase 4: output projection (this core's 256 columns) ----
            for qc in range(QC):
                ysb = xp.tile([128, CIN, 512], bf16, tag="xT", name="ysb")
                agf = ag_out[qc][:].rearrange("g c t -> (g c) t") \
                                   .rearrange("(o p) t -> p o t", p=128)
                for ci in range(CIN):
                    eng = nc.sync if ci % 2 == 0 else nc.scalar
                    eng.dma_start(ysb[:, ci, :], agf[:, ci, :])
                for tt in range(4):
                    pz = ps.tile([128, 2, 512], f32, tag="s", name="pz", bufs=3)
                    pz = pz[:, 0, 0:CL]
                    nc.tensor.matmul(pz, ones128[:], bp_sb[:],
                                     start=True, stop=False)
                    for ci in range(CIN):
                        nc.tensor.matmul(
                            pz, ysb[:, ci, tt * 128:(tt + 1) * 128],
                            wp_sb[:, ci, :],
                            start=False, stop=(ci == CIN - 1))
                    zs = work.tile([128, CL], f32, tag="zs")
                    nc.vector.tensor_copy(zs[:], pz)
                    t0 = qc * 512 + tt * 128
                    nc.scalar.dma_start(z[t0:t0 + 128, :], zs[:])

    nc.compile()
    return nc


def _get_program():
    if "nc" not in _prog_cache:
        _prog_cache["nc"] = _build_program()
    return _prog_cache["nc"]


def _pmajor(a2d):
    """[C, N] -> [128, C//128, N] partition-major contiguous."""
    Cdim, N = a2d.shape
    return np.ascontiguousarray(
        a2d.reshape(CIN, 128, N).transpose(1, 0, 2))


def kernel(x, Wq, bq, Wk, bk, Wv, bv, Wp, bp, resolve_level):
    import ml_dtypes
    from concourse.bass_utils import run_bass_kernel_spmd

    bfl = ml_dtypes.bfloat16
    nc = _get_program()

    x = np.asarray(x, np.float32)
    rl = np.asarray(resolve_level, np.float32).reshape(1, 1)

    xP_b = [_pmajor(np.ascontiguousarray(x[b].T).astype(bfl))
            for b in range(B)]
    in_maps = []
    for c in range(NCORES):
        b, hg = c // 4, c % 4
        cs = slice(hg * CL, (hg + 1) * CL)
        in_maps.append({
            "xP": xP_b[b],
            "wqP": _pmajor(np.asarray(Wq, np.float32)[cs, :].T.astype(bfl)),
            "wkP": _pmajor(np.asarray(Wk, np.float32)[cs, :].T.astype(bfl)),
            "wvP": _pmajor(np.asarray(Wv, np.float32)[cs, :].T.astype(bfl)),
            "wpP": _pmajor(np.asarray(Wp, np.float32)[cs, :].T.astype(bfl)),
            "bqC": np.ascontiguousarray(
                np.asarray(bq, np.float32)[cs].reshape(NP, 128).T),
            "bkC": np.ascontiguousarray(
                np.asarray(bk, np.float32)[cs].reshape(NP, 128).T),
            "bv": np.asarray(bv, np.float32)[cs].reshape(1, CL).astype(bfl),
            "bp": np.asarray(bp, np.float32)[cs].reshape(1, CL).astype(bfl),
            "rlv": rl,
            "ones_d": np.ones((1, 512), bfl),
        })

    res = run_bass_kernel_spmd(nc, in_maps, core_ids=list(range(NCORES)))

    out = np.empty((B, T, C), np.float32)
    for c in range(NCORES):
        b, hg = c // 4, c % 4
        out[b, :, hg * CL:(hg + 1) * CL] = res.results[c]["z"]
    return out


# revision 17
# speedup vs baseline: 1.8962x; 1.0666x over previous
"""Trainium2 Bass kernel for nn_DynamicResolutionAttention.

B=2, T=2048, C=1024, H=16 heads, head_dim=64.
  q/k/v = x @ W{q,k,v}.T + b     (per-head views)
  attn  = softmax(q k^T / sqrt(hd) * (0.5 + 0.5*resolve))
  y     = attn @ v ; out = y @ Wp.T + bp

Sharding (8 cores): core c = (batch b=c//4, head-group hg=c%4, 4 heads each).
Per core: QKV projections for its 4 heads (transpose-free d-major layouts,
host-pretransposed x^T / W^T in partition-major order for wide DMA lines),
k-major scores S^T = K_h Q_h^T so softmax denominators come from an appended
ones-column on V and no on-chip transpose is ever needed, exp on the Scalar
engine (softmax temperature pre-folded into Q), AllGather of y^T within each
batch's 4 cores (split into two token-halves so the first gather overlaps the
second half of attention), then each core computes the output projection
restricted to its own 256 output channels (column split -> no all-reduce).
Host reassembles [B,T,C] from the 8 [T,256] column slices.

Matmul operands are bf16 (fp32 PSUM accumulation); softmax statistics fp32.
"""

import sys

for _p in ("/opt/trn_rl_repo",):
    if _p not in sys.path:
        sys.path.insert(0, _p)

import numpy as np

B, T, C, H = 2, 2048, 1024, 16
HD = C // H            # 64
NCORES = 8
HL = 4                 # heads per core
NP = HL // 2           # head pairs per core
CL = HL * HD           # 256 local channels
CIN = C // 128         # 8 contraction tiles
KT_TILES = T // 128    # 16
QC = T // 512          # 4 query chunks
TH = T // 2            # token half

_prog_cache = {}


def _build_program():
    import concourse.mybir as mybir
    import concourse.tile as tile
    from concourse import bacc

    f32 = mybir.dt.float32
    bf16 = mybir.dt.bfloat16

    nc = bacc.Bacc("TRN2", target_bir_lowering=False, debug=False,
                   num_devices=NCORES)

    # host-prearranged partition-major layouts (long contiguous DMA lines)
    xP = nc.dram_tensor("xP", [128, CIN, T], bf16, kind="ExternalInput")
    wqP = nc.dram_tensor("wqP", [128, CIN, CL], bf16, kind="ExternalInput")
    wkP = nc.dram_tensor("wkP", [128, CIN, CL], bf16, kind="ExternalInput")
    wvP = nc.dram_tensor("wvP", [128, CIN, CL], bf16, kind="ExternalInput")
    wpP = nc.dram_tensor("wpP", [128, CIN, CL], bf16, kind="ExternalInput")
    bqC = nc.dram_tensor("bqC", [128, NP], f32, kind="ExternalInput")
    bkC = nc.dram_tensor("bkC", [128, NP], f32, kind="ExternalInput")
    bv = nc.dram_tensor("bv", [1, CL], bf16, kind="ExternalInput")
    bp = nc.dram_tensor("bp", [1, CL], bf16, kind="ExternalInput")
    rlv = nc.dram_tensor("rlv", [1, 1], f32, kind="ExternalInput")
    ones_d = nc.dram_tensor("ones_d", [1, 512], bf16, kind="ExternalInput")
    z = nc.dram_tensor("z", [T, CL], f32, kind="ExternalOutput")

    with tile.TileContext(nc) as tc:
        with tc.tile_pool(name="const", bufs=1) as const, \
             tc.tile_pool(name="big", bufs=1) as big, \
             tc.tile_pool(name="xp", bufs=2) as xp, \
             tc.tile_pool(name="work", bufs=3) as work, \
             tc.tile_pool(name="ps", bufs=2, space="PSUM") as ps, \
             tc.tile_pool(name="dram", bufs=1, space="DRAM") as dram:

            # runtime softmax scale: (0.5 + 0.5*resolve) / sqrt(hd)
            st = const.tile([128, 1], f32)
            nc.sync.dma_start(st[:], rlv[:].to_broadcast((128, 1)))
            nc.vector.tensor_scalar(st[:], st[:], 0.0625, 0.0625,
                                    mybir.AluOpType.mult, mybir.AluOpType.add)

            ones512 = const.tile([1, 512], bf16)
            nc.sync.dma_start(ones512[:], ones_d[:])
            ones128 = const.tile([1, 128], bf16)
            nc.sync.dma_start(ones128[:], ones_d[:, 0:128])

            bqC_sb = const.tile([128, NP], f32)
            bkC_sb = const.tile([128, NP], f32)
            bv_sb = const.tile([1, CL], bf16)
            bp_sb = const.tile([1, CL], bf16)
            nc.sync.dma_start(bqC_sb[:], bqC[:])
            nc.sync.dma_start(bkC_sb[:], bkC[:])
            nc.sync.dma_start(bv_sb[:], bv[:])
            nc.sync.dma_start(bp_sb[:], bp[:])

            # x resident for phase 1: 8 parallel DMAs, 4KB lines
            xs = xp.tile([128, CIN, T], bf16, tag="xT")
            for ci in range(CIN):
                for hf in range(2):
                    eng = nc.sync if (ci * 2 + hf) % 2 == 0 else nc.scalar
                    eng.dma_start(xs[:, ci, hf * TH:(hf + 1) * TH],
                                  xP[:, ci, hf * TH:(hf + 1) * TH])

            wq_sb = big.tile([128, CIN, CL], bf16)
            wk_sb = big.tile([128, CIN, CL], bf16)
            wv_sb = big.tile([128, CIN, CL], bf16)
            wp_sb = big.tile([128, CIN, CL], bf16)
            for i, (w_sb, w_dram) in enumerate(
                    ((wq_sb, wqP), (wk_sb, wkP), (wv_sb, wvP))):
                eng = nc.scalar if i % 2 == 0 else nc.sync
                eng.dma_start(w_sb[:, 0:4, :], w_dram[:, 0:4, :])
                eng.dma_start(w_sb[:, 4:8, :], w_dram[:, 4:8, :])

            QTp = [big.tile([128, T], bf16, name=f"QT{p}") for p in range(NP)]
            KTp = [big.tile([128, T], bf16, name=f"KT{p}") for p in range(NP)]
            Vp = [big.tile([128, KT_TILES, 2, HD + 1], bf16, name=f"V{p}")
                  for p in range(NP)]
            for p in range(NP):
                nc.sync.dma_start(
                    Vp[p][:, :, :, HD].rearrange("p a b -> p (a b)"),
                    ones_d[0:1, 0:1].to_broadcast((128, KT_TILES * 2)))

            # ---- phase 1: QKV projections (pair 0 first for early attn) --
            def qk_proj(pair):
                pc = slice(pair * 128, (pair + 1) * 128)
                for w_sb, OUT, bc in ((wq_sb, QTp[pair], bqC_sb),
                                      (wk_sb, KTp[pair], bkC_sb)):
                    for ch in range(QC):
                        pm = ps.tile([128, 2, 512], f32, tag="s",
                                     name="pm", bufs=3)
                        pm = pm[:, 0, :]
                        for ci in range(CIN):
                            nc.tensor.matmul(
                                pm, w_sb[:, ci, pc],
                                xs[:, ci, ch * 512:(ch + 1) * 512],
                                start=(ci == 0), stop=(ci == CIN - 1))
                        dst = OUT[:, ch * 512:(ch + 1) * 512]
                        if OUT is QTp[pair]:
                            # (q + bias) * temperature
                            nc.vector.tensor_scalar(
                                dst, pm, bc[:, pair:pair + 1], st[:],
                                mybir.AluOpType.add, mybir.AluOpType.mult)
                        else:
                            nc.vector.tensor_scalar_add(
                                dst, pm, bc[:, pair:pair + 1])

            def v_proj():
                for tt in range(KT_TILES):
                    pv = ps.tile([128, 2, 512], f32, tag="s", name="pv", bufs=3)
                    pv = pv[:, 0, 0:CL]
                    nc.tensor.matmul(pv, ones128[:], bv_sb[:],
                                     start=True, stop=False)
                    for ci in range(CIN):
                        nc.tensor.matmul(
                            pv, xs[:, ci, tt * 128:(tt + 1) * 128],
                            wv_sb[:, ci, :],
                            start=False, stop=(ci == CIN - 1))
                    for p in range(NP):
                        nc.vector.tensor_copy(
                            Vp[p][:, tt, :, 0:HD],
                            pv[:, p * 128:(p + 1) * 128]
                            .rearrange("p (h d) -> p h d", h=2))

            qk_proj(0)
            v_proj()
            qk_proj(1)

            # wp loads overlap attention
            nc.scalar.dma_start(wp_sb[:, 0:4, :], wpP[:, 0:4, :])
            nc.scalar.dma_start(wp_sb[:, 4:8, :], wpP[:, 4:8, :])

            ag_in = [dram.tile([CL, 512], bf16, name=f"ag_in{i}")
                     for i in range(QC)]
            rec_d = dram.tile([16, 512], f32, name="rec_d")
            ag_out = [dram.tile([4, CL, 512], bf16, name=f"ag_out{i}")
                      for i in range(QC)]

            # ---- phase 2: attention (k-major S^T, ones-column denom) ----
            # qc outer so each token-quarter's AllGather can start early
            for qc in range(QC):
                qs = slice(qc * 512, (qc + 1) * 512)
                for h in range(HL):
                    pair, off = h // 2, (h % 2) * HD
                    hh = h % 2
                    QT_, KT_, V_ = QTp[pair], KTp[pair], Vp[pair]
                    py = ps.tile([HD + 1, 512], f32, tag="y", name="py")
                    for k2 in range(KT_TILES // 2):
                        pss = ps.tile([128, 2, 512], f32, tag="s",
                                      name="pss", bufs=3)
                        for j in range(2):
                            kt = k2 * 2 + j
                            nc.tensor.matmul(
                                pss[:, j, :],
                                KT_[off:off + HD,
                                    kt * 128:(kt + 1) * 128],
                                QT_[off:off + HD, qs],
                                start=True, stop=True)
                        pt = work.tile([128, 2, 512], bf16, tag="pt", bufs=6)
                        nc.scalar.activation(
                            pt[:], pss[:],
                            mybir.ActivationFunctionType.Exp)
                        for j in range(2):
                            kt = k2 * 2 + j
                            nc.tensor.matmul(
                                py[:], V_[:, kt, hh, :], pt[:, j, :],
                                start=(kt == 0),
                                stop=(kt == KT_TILES - 1))
                    rec = work.tile([1, 512], f32, tag="rec")
                    nc.vector.reciprocal(rec[:], py[HD:HD + 1, :])
                    slot = qc * HL + h
                    nc.sync.dma_start(rec_d[slot:slot + 1, :], rec[:])
                    pbs = work.tile([HD, 512], f32, tag="pbs")
                    nc.sync.dma_start(
                        pbs[:], rec_d[slot:slot + 1, :].to_broadcast((HD, 512)))
                    yt = work.tile([HD, 512], bf16, tag="yt")
                    nc.vector.tensor_mul(yt[:], py[0:HD, :], pbs[:])
                    nc.sync.dma_start(
                        ag_in[qc][h * HD:(h + 1) * HD, :], yt[:])

                # ---- phase 3: gather heads' y^T for this token quarter ----
                nc.gpsimd.collective_compute(
                    "AllGather", mybir.AluOpType.bypass,
                    replica_groups=[[0, 1, 2, 3], [4, 5, 6, 7]],
                    ins=[ag_in[qc].opt()], outs=[ag_out[qc].opt()])

            # ---- phase 4: output projection (this core's 256 columns) ----
            for qc in range(QC):
                ysb = xp.tile([128, CIN, 512], bf16, tag="xT", name="ysb")
                agf = ag_out[qc][:].rearrange("g c t -> (g c) t") \
                                   .rearrange("(o p) t -> p o t", p=128)
                for ci in range(CIN):
                    eng = nc.sync if ci % 2 == 0 else nc.scalar
                    eng.dma_start(
                        ysb[:, ci:ci + 1, :],
                        agf[:, ci:ci + 1, :])
                for tt in range(4):
                    pz = ps.tile([128, 2, 512], f32, tag="s", name="pz", bufs=3)
                    pz = pz[:, 0, 0:CL]
                    nc.tensor.matmul(pz, ones128[:], bp_sb[:],
                                     start=True, stop=False)
                    for ci in range(CIN):
                        nc.tensor.matmul(
                            pz, ysb[:, ci, tt * 128:(tt + 1) * 128],
                            wp_sb[:, ci, :],
                            start=False, stop=(ci == CIN - 1))
                    zs = work.tile([128, CL], f32, tag="zs")
                    nc.vector.tensor_copy(zs[:], pz)
                    t0 = qc * 512 + tt * 128
                    nc.scalar.dma_start(z[t0:t0 + 128, :], zs[:])

    nc.compile()
    return nc


def _get_program():
    if "nc" not in _prog_cache:
        _prog_cache["nc"] = _build_program()
    return _prog_cache["nc"]


def _pmajor(a2d):
    """[C, N] -> [128, C//128, N] partition-major contiguous."""
    Cdim, N = a2d.shape
    return np.ascontiguousarray(
        a2d.reshape(CIN, 128, N).transpose(1, 0, 2))


def kernel(x, Wq, bq, Wk, bk, Wv, bv, Wp, bp, resolve_level):
    import ml_dtypes
    from concourse.bass_utils import run_bass_kernel_spmd

    bfl = ml_dtypes.bfloat16
    nc = _get_program()

    x = np.asarray(x, np.float32)
    rl = np.asarray(resolve_level, np.float32).reshape(1, 1)

    xP_b = [_pmajor(np.ascontiguousarray(x[b].T).astype(bfl))
            for b in range(B)]
    in_maps = []
    for c in range(NCORES):
        b, hg = c // 4, c % 4
        cs = slice(hg * CL, (hg + 1) * CL)
        in_maps.append({
            "xP": xP_b[b],
            "wqP": _pmajor(np.asarray(Wq, np.float32)[cs, :].T.astype(bfl)),
            "wkP": _pmajor(np.asarray(Wk, np.float32)[cs, :].T.astype(bfl)),
            "wvP": _pmajor(np.asarray(Wv, np.float32)[cs, :].T.astype(bfl)),
            "wpP": _pmajor(np.asarray(Wp, np.float32)[cs, :].T.astype(bfl)),
            "bqC": np.ascontiguousarray(
                np.asarray(bq, np.float32)[cs].reshape(NP, 128).T),
            "bkC": np.ascontiguousarray(
                np.asarray(bk, np.float32)[cs].reshape(NP, 128).T),
            "bv": np.asarray(bv, np.float32)[cs].reshape(1, CL).astype(bfl),
            "bp": np.asarray(bp, np.float32)[cs].reshape(1, CL).astype(bfl),
            "rlv": rl,
            "ones_d": np.ones((1, 512), bfl),
        })

    res = run_bass_kernel_spmd(nc, in_maps, core_ids=list(range(NCORES)))

    out = np.empty((B, T, C), np.float32)
    for c in range(NCORES):
        b, hg = c // 4, c % 4
        out[b, :, hg * CL:(hg + 1) * CL] = res.results[c]["z"]
    return out
